# revision 63
# baseline (speedup 1.0000x reference)
"""Trainium2 Bass kernel for one transformer block (B=2, T=2048, C=768, H=12,
inner=3072, fp32 I/O, causal, post-norm residual).

Sharding: 8 cores, token-interleaved. Core c handles batch c//4, tokens
p::4 (p = c%4) of that batch — every core runs the IDENTICAL program
(SPMD); causality is data-driven via per-core mask tensors.

v2 vs baseline:
- all GEMMs in bf16 (weights pre-cast on host; activations cast on write)
  -> half the HBM traffic, no f32r N>=256 floor, half LDWEIGHTS time.
- scores contraction K=64 directly (no zero-padded q rows / zer DMA).
- kc>=8 score regions packed tightly (no 256-col floor waste); one extra
  128-wide slot rides in the kc2=3 tile so the 8 leftover chunks fit a
  single [128,1024] 2-bank psum tile.
- mask-mults alternate DVE/Pool to halve DVE load.
- softmax denominators: per-chunk [2,TQ] + fast approx reciprocal + one
  K=2 broadcast matmul (replaces [65,*] reciprocal + per-head broadcasts).
- layernorm apply via two rank<=2 broadcast matmuls (P1 = s*istd,
  P2 = s*mean*istd - b) -> 2 DVE ops per chunk instead of 3.
- wo/w1/w2 prefetched during attention.
"""

import sys

if "/opt/trn_rl_repo" not in sys.path:
    sys.path.insert(0, "/opt/trn_rl_repo")

import numpy as np
import ml_dtypes

import concourse.bacc as bacc
import concourse.mybir as mybir
import concourse.tile as tile
from concourse.bass_utils import run_bass_kernel_spmd

F32 = mybir.dt.float32
F32R = mybir.dt.float32r
BF16 = mybir.dt.bfloat16
ACTF = mybir.ActivationFunctionType

B, T, C = 2, 2048, 768
H, DH = 12, 64
IN = 3072
CC = C // 128          # 6 channel chunks
TBN = T // 512         # 4 token blocks of full seq
TQ = 512               # tokens per core
KCN = T // 128         # 16 k-chunks
ICN = IN // 128        # 24 inner chunks
EPS = 1e-4
SCALE = 1.0 / np.sqrt(DH)

# param pack order in "prk" [128, CC, 8]
P_BQ, P_BK, P_BO, P_B2, P_L1S, P_L1B, P_L2S, P_L2B = range(8)

# leftover score chunks (kc >= 8, except kc=12 which rides in the kc2=3
# tile at cols [800:928]): (kc, col0, width); widths bin-packed so no
# matmul output crosses a 2KB psum bank boundary.
SA_SLOTS = [(8, 0, 256), (9, 256, 224), (15, 480, 32),
            (10, 512, 192), (11, 704, 160), (13, 864, 96), (14, 960, 64)]


def _build_nc():
    nc = bacc.Bacc("TRN2", target_bir_lowering=False, debug=False,
                   enable_asserts=False, num_devices=8)
    d = {}
    d["xt"] = nc.dram_tensor("xt", [128, TBN, CC, 512], BF16,
                             kind="ExternalInput").ap()
    d["xtqh"] = nc.dram_tensor("xtqh", [128, CC, TQ], BF16,
                               kind="ExternalInput").ap()
    d["xtq"] = nc.dram_tensor("xtq", [128, CC, TQ], F32R,
                              kind="ExternalInput").ap()
    for w in ("wq", "wk", "wv", "wo"):
        d[w] = nc.dram_tensor(w, [128, CC, C], BF16,
                              kind="ExternalInput").ap()
    d["w1"] = nc.dram_tensor("w1", [128, 4, CC, C], BF16,
                             kind="ExternalInput").ap()
    d["w2"] = nc.dram_tensor("w2", [128, 4, CC, C], BF16,
                             kind="ExternalInput").ap()
    d["prk"] = nc.dram_tensor("prk", [128, CC, 8], F32, kind="ExternalInput").ap()
    d["b1p"] = nc.dram_tensor("b1p", [128, ICN], F32, kind="ExternalInput").ap()
    d["bvb"] = nc.dram_tensor("bvb", [128, C], F32, kind="ExternalInput").ap()
    d["msk"] = nc.dram_tensor("msk", [128, 32], BF16, kind="ExternalInput").ap()
    d["ones"] = nc.dram_tensor("ones", [128, TQ], F32R, kind="ExternalInput").ap()
    d["selm"] = nc.dram_tensor("selm", [65, 3, 128], F32R, kind="ExternalInput").ap()
    d["lnz"] = nc.dram_tensor("lnz", [2, 65, TQ], F32R, kind="ExternalInput").ap()
    d["lnt"] = nc.dram_tensor("lnt", [2, 65, C], F32R, kind="ExternalInput").ap()
    d["outT"] = nc.dram_tensor("outT", [C, TQ], F32, kind="ExternalOutput").ap()

    with tile.TileContext(nc) as tc:
        _emit(nc, tc, d)
    nc.finalize()
    return nc


def _ln_prefetch(nc, d, pool, tagp):
    """Allocate + zero-fill the LN broadcast rhs tiles early (off the
    critical chain): lnr1 all-zero, lnr2 row32 = ones."""
    lnr1 = pool.tile([65, TQ], F32R, name="lnr1", tag=tagp + "r1")
    nc.gpsimd.dma_start(out=lnr1[:], in_=d["lnz"][0, :, :])
    lnr2 = pool.tile([65, TQ], F32R, name="lnr2", tag=tagp + "r2")
    nc.gpsimd.dma_start(out=lnr2[:], in_=d["lnz"][1, :, :])
    return lnr1, lnr2


def _ln_bcast(nc, pool, eps_sb, lnr1, lnr2, mean, ex2, tagp):
    """Fill lnr1 (row0 = istd) and lnr2 (row0 = mean*istd, row32 = ones)
    so the LN broadcasts run as K=65 selector matmuls at full PE rate.
    istd = exp(-0.5*ln(var+eps)) on ACT (stays in the natural_log_exp
    table; no Sqrt table, no DVE reciprocal)."""
    n = float(C)
    m2 = pool.tile([1, TQ], F32, name="ln_m2", tag=tagp + "m2")
    nc.vector.tensor_mul(m2[:], mean[:], mean[:])
    dv = pool.tile([1, TQ], F32, name="ln_d", tag=tagp + "d")
    nc.vector.tensor_sub(dv[:], ex2[:], m2[:])
    lnv = pool.tile([1, TQ], F32, name="ln_lnv", tag=tagp + "lnv")
    nc.scalar.activation(lnv[:], dv[:], ACTF.Ln,
                         scale=n / (n - 1.0), bias=eps_sb[:])
    with nc.allow_low_precision(reason="f32r matmul operand"):
        nc.scalar.activation(lnr1[0:1, :], lnv[:], ACTF.Exp, scale=-0.5)
        nc.vector.tensor_mul(lnr2[0:1, :], mean[:], lnr1[0:1, :])


def _emit(nc, tc, d):
    # ---- persistent constants ------------------------------------------
    const = tc.alloc_tile_pool(name="const", bufs=1, side="left")
    ones_sb = const.tile([128, TQ], F32R, name="ones_sb")
    selm_sb = const.tile([65, 3, 128], F32R, name="selm_sb")
    lnt1_sb = const.tile([65, C], F32R, name="lnt1_sb")
    lnt2_sb = const.tile([65, C], F32R, name="lnt2_sb")
    eps_sb = const.tile([1, 1], F32, name="eps_sb")
    prk_sb = const.tile([128, CC, 8], F32, name="prk_sb")
    b1p_sb = const.tile([128, ICN], F32, name="b1p_sb")
    bvb_sb = const.tile([128, C], F32, name="bvb_sb")

    def prm(cc, pi):
        return prk_sb[:, cc, pi].unsqueeze(-1)  # [128,1]

    xtq_pool = tc.alloc_tile_pool(name="xtq", bufs=1, side="left")
    xtqh_pk = xtq_pool.tile([128, CC, TQ], BF16, name="xtqh_pk")
    nc.sync.dma_start(out=xtqh_pk[:], in_=d["xtqh"][:])
    xtqh_sb = [xtqh_pk[:, cc, :] for cc in range(CC)]
    # prk first (tiny, needed by q bias add); bigger phase-B/C consts are
    # emitted after the attention-critical weight streams
    nc.vector.memset(eps_sb[:], float(EPS))
    nc.gpsimd.dma_start(out=prk_sb[:], in_=d["prk"][:])
    xtq_pk = xtq_pool.tile([128, CC, TQ], F32R, name="xtq_pk")
    xtq_sb = [xtq_pk[:, cc, :] for cc in range(CC)]

    qT_pool = tc.alloc_tile_pool(name="qTp", bufs=1, side="left")
    qz_sb = [qT_pool.tile([128, TQ], BF16, name=f"qz{h}") for h in range(H)]
    for h in range(H):
        ro = (h % 2) * DH
        with nc.allow_low_precision(reason="zero fill"):
            nc.vector.memset(qz_sb[h][(DH - ro):(128 - ro), :], 0.0)

    kv_pool = tc.alloc_tile_pool(name="kvp", bufs=1, side="left")
    kT_sb = [kv_pool.tile([128, T], BF16, name=f"kT{cc}") for cc in range(CC)]
    v_sb = [kv_pool.tile([128, H, DH + 1], BF16, name=f"v{tch}")
            for tch in range(KCN)]

    # ==================== phase A: QKV ==================================
    with tc.tile_pool(name="wstr", bufs=6, side="right") as wpool, \
         tc.tile_pool(name="xts", bufs=2, side="right") as xt_pool, \
         tc.tile_pool(name="pqkv", bufs=2, space="PSUM") as pqkv:

        wq_ha = wpool.tile([128, 3, C], BF16, name="w_t", tag="w")
        nc.scalar.dma_start(out=wq_ha[:], in_=d["wq"][:, 0:3, :])
        wq_hb = wpool.tile([128, 3, C], BF16, name="w_t", tag="w")
        nc.scalar.dma_start(out=wq_hb[:], in_=d["wq"][:, 3:6, :])
        wq_sb = [(wq_ha if cc < 3 else wq_hb)[:, cc % 3, :]
                  for cc in range(CC)]
        wk_ha = wpool.tile([128, 3, C], BF16, name="w_t", tag="w")
        nc.gpsimd.dma_start(out=wk_ha[:], in_=d["wk"][:, 0:3, :])
        wk_hb = wpool.tile([128, 3, C], BF16, name="w_t", tag="w")
        nc.gpsimd.dma_start(out=wk_hb[:], in_=d["wk"][:, 3:6, :])
        wk_sb = [(wk_ha if cc < 3 else wk_hb)[:, cc % 3, :]
                  for cc in range(CC)]
        # q^T = Wq^T x_q^T + bq; head h occupies rows (h%2)*64..+64 of qz[h]
        for mc in range(CC):
            ps = pqkv.tile([128, TQ], F32, name="ps_q", tag="pq", bufs=4)
            for kc in range(CC):
                nc.tensor.matmul(ps[:], wq_sb[kc][:, mc * 128:(mc + 1) * 128],
                                 xtqh_sb[kc][:],
                                 start=(kc == 0), stop=(kc == CC - 1))
            with nc.allow_low_precision(reason="bf16 activations"):
                nc.vector.tensor_scalar_add(
                    qz_sb[2 * mc][0:DH, :], ps[0:DH, :],
                    prm(mc, P_BQ)[0:DH, :])
                nc.vector.tensor_scalar_add(
                    qz_sb[2 * mc + 1][DH:128, :], ps[DH:128, :],
                    prm(mc, P_BQ)[DH:128, :])

        wv_ha = wpool.tile([128, 3, C], BF16, name="w_t", tag="w")
        nc.gpsimd.dma_start(out=wv_ha[:], in_=d["wv"][:, 0:3, :])
        wv_hb = wpool.tile([128, 3, C], BF16, name="w_t", tag="w")
        nc.gpsimd.dma_start(out=wv_hb[:], in_=d["wv"][:, 3:6, :])
        nc.gpsimd.dma_start(out=bvb_sb[:], in_=d["bvb"][:])
        nc.gpsimd.dma_start(out=ones_sb[:], in_=d["ones"][:])
        nc.gpsimd.dma_start(out=selm_sb[:], in_=d["selm"][:])
        wv_sb = [(wv_ha if cc < 3 else wv_hb)[:, cc % 3, :]
                  for cc in range(CC)]

        for tb in range(TBN):
            xt_pk = xt_pool.tile([128, CC, 512], BF16, name="xt_t", tag="xt")
            eng = nc.sync if tb % 2 == 0 else nc.scalar
            eng.dma_start(out=xt_pk[:], in_=d["xt"][:, tb, :, :])
            xt_blk = [xt_pk[:, cc, :] for cc in range(CC)]
            # k^T columns of this block
            for mc in range(CC):
                ps = pqkv.tile([128, 512], F32, name="ps_k", tag="pq",
                               bufs=4)
                for kc in range(CC):
                    nc.tensor.matmul(ps[:],
                                     wk_sb[kc][:, mc * 128:(mc + 1) * 128],
                                     xt_blk[kc][:],
                                     start=(kc == 0), stop=(kc == CC - 1))
                with nc.allow_low_precision(reason="bf16 activations"):
                    nc.vector.tensor_scalar_add(
                        kT_sb[mc][:, tb * 512:(tb + 1) * 512], ps[:],
                        prm(mc, P_BK))
            # v rows (natural layout), 4 chunks of 128 tokens each
            for tci in range(4):
                tch = tb * 4 + tci
                ps1 = pqkv.tile([128, 512], F32, name="ps_v1", tag="pv1")
                ps2 = pqkv.tile([128, 256], F32, name="ps_v2", tag="pv2")
                for kc in range(CC):
                    xsl = xt_blk[kc][:, tci * 128:(tci + 1) * 128]
                    nc.tensor.matmul(ps1[:], xsl, wv_sb[kc][:, 0:512],
                                     start=(kc == 0), stop=(kc == CC - 1))
                    nc.tensor.matmul(ps2[:], xsl, wv_sb[kc][:, 512:C],
                                     start=(kc == 0), stop=(kc == CC - 1))
                vt = v_sb[tch]
                with nc.allow_low_precision(reason="bf16 activations"):
                    nc.vector.tensor_add(
                        vt[:, 0:8, 0:DH],
                        ps1[:].rearrange("p (h d) -> p h d", d=DH),
                        bvb_sb[:, 0:512].rearrange("p (h d) -> p h d", d=DH))
                    nc.vector.tensor_add(
                        vt[:, 8:H, 0:DH],
                        ps2[:].rearrange("p (h d) -> p h d", d=DH),
                        bvb_sb[:, 512:C].rearrange("p (h d) -> p h d", d=DH))
                    nc.vector.tensor_copy(vt[:, :, DH], ones_sb[:, 0:H])

    # ==================== phase B: attention ============================
    ctxT_pool = tc.alloc_tile_pool(name="ctxTp", bufs=1, side="right")
    ctxT_sb = [ctxT_pool.tile([128, TQ], BF16, name=f"ctxT{cc}")
               for cc in range(CC)]
    # long-lived weight pools, prefetched during attention
    wo_pool = tc.alloc_tile_pool(name="wop", bufs=1, side="right")
    w1pool = tc.alloc_tile_pool(name="w1pool", bufs=2, side="right")
    w2pool = tc.alloc_tile_pool(name="w2pool", bufs=2, side="right")

    with tc.tile_pool(name="mskp", bufs=1, side="right") as mpool, \
         tc.tile_pool(name="attnp", bufs=6, side="right") as apool, \
         tc.tile_pool(name="psc", bufs=1, space="PSUM") as psc, \
         tc.tile_pool(name="pctx", bufs=1, space="PSUM") as pctx:
        mskb_sb = mpool.tile([128, 32], BF16, name="mskb_sb")
        nc.gpsimd.dma_start(out=mskb_sb[:], in_=d["msk"][:])
        nc.gpsimd.dma_start(out=xtq_pk[:], in_=d["xtq"][:])
        nc.gpsimd.dma_start(out=lnt1_sb[:], in_=d["lnt"][0, :, :])
        nc.gpsimd.dma_start(out=lnt2_sb[:], in_=d["lnt"][1, :, :])
        nc.gpsimd.dma_start(out=b1p_sb[:], in_=d["b1p"][:])
        # prefetch wo + first two w1 blocks + first w2 tiles
        wo_pk = wo_pool.tile([128, CC, C], BF16, name="wo_pk")
        nc.gpsimd.dma_start(out=wo_pk[:], in_=d["wo"][:])
        wo_sb = [wo_pk[:, cc, :] for cc in range(CC)]
        w1blk = {}
        for jb in range(2):
            t = w1pool.tile([128, CC, C], BF16, name="w1_t", tag="w1")
            nc.gpsimd.dma_start(out=t[:], in_=d["w1"][:, jb, :, :])
            w1blk[jb] = [t[:, kc, :] for kc in range(CC)]
        w2g = {}
        for gg in range(2):
            t = w2pool.tile([128, CC, C], BF16, name="w2_t", tag="w2")
            nc.gpsimd.dma_start(out=t[:], in_=d["w2"][:, gg, :, :])
            w2g[gg] = t

        # pre-allocate + zero all four denominator tiles up front so the
        # per-group memset never sits in the DVE queue ahead of mask work
        dn_tiles = []
        for gidx in range(4):
            dnt = apool.tile([65, TQ], F32, name="dn", tag="dn")
            nc.vector.memset(dnt[:], 1.0)
            dn_tiles.append(dnt)
        pending = []
        for h in range(H):
            cc, ro = h // 2, (h % 2) * DH
            kTh = kT_sb[cc]
            qzh = qz_sb[h]
            ctx_ps = pctx.tile([DH + 1, TQ], F32, name="ctx_ps", tag="ctx",
                               bufs=1)
            ctx_started = False

            def score_group(msk_i, mm_list, exp_lo, exp_hi, ctx_list,
                            last=False):
                if pending:
                    pending.pop(0)()
                """mm_list: (out_lo, out_hi, kc, q_lo); ctx_list: (kc, c0, w)."""
                nonlocal ctx_started
                ps = psc.tile([128, 1024], F32, name="ps_s", tag="s",
                              bufs=3)
                for (o_lo, o_hi, kc, q_lo) in mm_list:
                    nc.tensor.matmul(
                        ps[:, o_lo:o_hi],
                        kTh[:, kc * 128:(kc + 1) * 128],
                        qzh[:, q_lo:TQ],
                        start=True, stop=True)
                et = apool.tile([128, 1024], BF16, name="et", tag="e",
                                bufs=10)
                nc.scalar.activation(et[:, exp_lo:exp_hi],
                                     ps[:, exp_lo:exp_hi], ACTF.Exp,
                                     scale=float(SCALE))
                # causality: only the first 32 columns of each slot are
                # partially masked (boundary); the pattern k<=4j+p is the
                # same for every chunk. Narrow multiplies on the idle Pool
                # engine keep DVE free.
                for (kc, c0, w) in ctx_list:
                    nc.vector.tensor_mul(et[:, c0:c0 + 32],
                                         et[:, c0:c0 + 32], mskb_sb[:])
                for i, (kc, c0, w) in enumerate(ctx_list):
                    nc.tensor.matmul(ctx_ps[:, TQ - w:], v_sb[kc][:, h, :],
                                     et[:, c0:c0 + w],
                                     start=(not ctx_started),
                                     stop=(last and i == len(ctx_list) - 1))
                    ctx_started = True

            for kc2 in range(4):
                s0, sl1 = 64 * kc2, 64 * kc2 + 32
                mm = [(s0, TQ, 2 * kc2, s0),
                      (TQ, 2 * TQ - sl1, 2 * kc2 + 1, sl1)]
                ctxl = [(2 * kc2, s0, TQ - s0), (2 * kc2 + 1, TQ, TQ - sl1)]
                hi = 2 * TQ - sl1
                if kc2 == 3:
                    mm.append((800, 928, 12, 384))
                    ctxl.append((12, 800, 128))
                    hi = 928
                score_group(kc2, mm, s0, hi, ctxl)
            score_group(4, [(c0, c0 + w, kc, TQ - w)
                            for (kc, c0, w) in SA_SLOTS],
                        0, 1024, SA_SLOTS, last=True)

            # evict: unnormalized ctx rows + denominator row. Denominators
            # for 4 heads land on rows 0/32/64/96 of one tile so a single
            # (expensive) DVE reciprocal serves all four.
            with nc.allow_low_precision(reason="bf16 activations"):
                nc.vector.tensor_copy(ctxT_sb[cc][ro:ro + DH, :],
                                      ctx_ps[0:DH, :])
            dn = dn_tiles[h // 3]
            dro = (h % 3) * 32
            nc.vector.tensor_copy(dn[dro:dro + 1, :],
                                  ctx_ps[DH:DH + 1, :])
            if h % 3 == 2:
                rcp = apool.tile([65, TQ], F32R, name="rcp", tag="rcp")

                def _rchunk(q0, dn=dn, rcp=rcp):
                    with nc.allow_low_precision(reason="f32r operand"):
                        nc.vector.reciprocal(rcp[:, q0:q0 + 128],
                                             dn[:, q0:q0 + 128])
                pending.extend(
                    (lambda q0=q0: _rchunk(q0)) for q0 in range(0, TQ, 128))

                def _norm(j, h=h, rcp=rcp):
                    hh = h - 2 + j
                    cc2, ro2 = hh // 2, (hh % 2) * DH
                    pb = pctx.tile([64, TQ], F32, name="pb", tag="bc",
                                   bufs=1)
                    nc.tensor.matmul(pb[0:DH, :], selm_sb[:, j, 0:DH],
                                     rcp[:], start=True, stop=True)
                    with nc.allow_low_precision(reason="bf16 activations"):
                        nc.vector.tensor_mul(
                            ctxT_sb[cc2][ro2:ro2 + DH, :],
                            ctxT_sb[cc2][ro2:ro2 + DH, :], pb[0:DH, :])
                pending.extend(
                    (lambda j=j: _norm(j)) for j in range(3))
        for op in pending:
            op()

    kv_pool.release()
    qT_pool.release()

    import os
    if os.environ.get("KTRUNC") == "B":
        with tc.tile_pool(name="trnc", bufs=1, side="right") as tp:
            for cc in range(CC):
                ot = tp.tile([128, TQ], F32, name=f"to{cc}", tag=f"to{cc}")
                nc.vector.tensor_copy(ot[:], xtq_sb[cc][:])
                nc.sync.dma_start(out=d["outT"][cc * 128:(cc + 1) * 128, :],
                                  in_=ot[:])
        w2pool.release()
        w1pool.release()
        wo_pool.release()
        ctxT_pool.release()
        xtq_pool.release()
        const.release()
        return

    # ==================== phase C: Wo + residual + LN1 ==================
    hT_holder = {}
    with tc.tile_pool(name="cpool", bufs=2, side="right") as cpool, \
         tc.tile_pool(name="r1pool", bufs=1, side="right") as r1pool:
        lnr1, lnr2 = _ln_prefetch(nc, d, cpool, "l1")
        r1_sb = [r1pool.tile([128, TQ], F32R, name=f"r1{cc}")
                 for cc in range(CC)]
        with tc.tile_pool(name="pao", bufs=2, space="PSUM") as pao, \
             tc.tile_pool(name="pst", bufs=2, space="PSUM") as pst:
            ps_sum = pst.tile([1, TQ], F32, name="ps_sum", tag="st")
            ps_sq = pst.tile([1, TQ], F32, name="ps_sq", tag="st")
            for mc in range(CC):
                ps = pao.tile([128, TQ], F32, name="ps_ao", tag="ao")
                for kc in range(CC):
                    nc.tensor.matmul(ps[:],
                                     wo_sb[kc][:, mc * 128:(mc + 1) * 128],
                                     ctxT_sb[kc][:],
                                     start=(kc == 0), stop=(kc == CC - 1))
                nc.vector.scalar_tensor_tensor(
                    r1_sb[mc][:], ps[:], prm(mc, P_BO), xtq_sb[mc][:],
                    mybir.AluOpType.add, mybir.AluOpType.add)
                nc.tensor.matmul(ps_sum[:], ones_sb[:, 0:1], r1_sb[mc][:],
                                 start=(mc == 0), stop=(mc == CC - 1))
                sq = cpool.tile([128, TQ], F32R, name="sq", tag="sq")
                nc.scalar.activation(sq[:], r1_sb[mc][:], ACTF.Square)
                nc.tensor.matmul(ps_sq[:], ones_sb[:, 0:1], sq[:],
                                 start=(mc == 0), stop=(mc == CC - 1))
            n = float(C)
            mean1 = cpool.tile([1, TQ], F32R, name="l1mean", tag="l1mean")
            nc.scalar.activation(mean1[:], ps_sum[:], ACTF.Copy, scale=1.0 / n)
            ex21 = cpool.tile([1, TQ], F32, name="l1ex2", tag="l1ex2")
            nc.scalar.activation(ex21[:], ps_sq[:], ACTF.Copy, scale=1.0 / n)
        xtq_pool.release()
        hT_pool = tc.alloc_tile_pool(name="hTp", bufs=1, side="left")
        hT_sb = [hT_pool.tile([128, TQ], BF16, name=f"hT{cc}")
                 for cc in range(CC)]
        hT_holder["pool"] = hT_pool
        _ln_bcast(nc, cpool, eps_sb, lnr1, lnr2, mean1, ex21, "l1")
        with tc.tile_pool(name="pbc2", bufs=2, space="PSUM") as pbc2:
            for cc in range(CC):
                csl = slice(cc * 128, (cc + 1) * 128)
                pb = pbc2.tile([128, 2 * TQ], F32, name="lnpb", tag="bc")
                nc.tensor.matmul(pb[:, 0:TQ], lnt1_sb[:, csl],
                                 lnr1[:], start=True, stop=True)
                nc.tensor.matmul(pb[:, TQ:], lnt1_sb[:, csl],
                                 lnr2[:], start=True, stop=True)
                t1 = cpool.tile([128, TQ], F32, name="ln_t1", tag="lnt1")
                nc.vector.tensor_mul(t1[:], r1_sb[cc][:], pb[:, 0:TQ])
                with nc.allow_low_precision(reason="bf16 activations"):
                    nc.vector.tensor_sub(hT_sb[cc][:], t1[:], pb[:, TQ:])

    import os as _os
    if _os.environ.get("KTRUNC") == "C":
        with tc.tile_pool(name="trnc2", bufs=1, side="right") as tp:
            for cc in range(CC):
                ot = tp.tile([128, TQ], F32, name=f"tc{cc}", tag=f"tc{cc}")
                nc.vector.tensor_copy(ot[:], hT_sb[cc][:])
                nc.sync.dma_start(out=d["outT"][cc * 128:(cc + 1) * 128, :],
                                  in_=ot[:])
        hT_holder["pool"].release()
        w2pool.release()
        w1pool.release()
        wo_pool.release()
        ctxT_pool.release()
        const.release()
        return

    # ==================== phase D: MLP + residual + LN2 =================
    with tc.tile_pool(name="dpool", bufs=3, side="right") as dpool, \
         tc.tile_pool(name="r2pool", bufs=1, side="right") as r2pool:

        r2_sb = [r2pool.tile([128, TQ], F32R, name=f"r2{cc}")
                 for cc in range(CC)]
        lnr1b, lnr2b = _ln_prefetch(nc, d, dpool, "l2")
        with tc.tile_pool(name="pfc2", bufs=1, space="PSUM") as pfc2:
            ps_m = [pfc2.tile([128, TQ], F32, name=f"ps_m{mc}", tag=f"m{mc}")
                    for mc in range(CC)]
            with tc.tile_pool(name="pfc1", bufs=2, space="PSUM") as pfc1:
                for kc2 in range(ICN):
                    jb = kc2 // CC
                    ps1 = pfc1.tile([128, TQ], F32, name="ps1", tag="f1")
                    co = (kc2 % CC) * 128
                    for kc in range(CC):
                        nc.tensor.matmul(
                            ps1[:], w1blk[jb][kc][:, co:co + 128],
                            hT_sb[kc][:],
                            start=(kc == 0), stop=(kc == CC - 1))
                    g = dpool.tile([128, TQ], BF16, name="g", tag="g")
                    with nc.allow_low_precision(reason="bf16 activations"):
                        nc.scalar.activation(g[:], ps1[:],
                                             ACTF.Gelu_apprx_tanh,
                                             bias=b1p_sb[:, kc2].unsqueeze(-1))
                    w2t = w2g[kc2 // CC][:, kc2 % CC, :]
                    for mc in range(CC):
                        nc.tensor.matmul(ps_m[mc][:],
                                         w2t[:, mc * 128:(mc + 1) * 128],
                                         g[:], start=(kc2 == 0),
                                         stop=(kc2 == ICN - 1))
                    # prefetches: issued only after this iteration's readers
                    # exist, so ring-reuse deps point at emitted instructions
                    if kc2 % CC == CC - 1 and jb + 2 <= 3:
                        t = w1pool.tile([128, CC, C], BF16, name="w1_t",
                                        tag="w1")
                        nc.gpsimd.dma_start(out=t[:],
                                            in_=d["w1"][:, jb + 2, :, :])
                        w1blk[jb + 2] = [t[:, kc, :] for kc in range(CC)]
                        t2 = w2pool.tile([128, CC, C], BF16, name="w2_t",
                                         tag="w2")
                        nc.gpsimd.dma_start(out=t2[:],
                                            in_=d["w2"][:, jb + 2, :, :])
                        w2g[jb + 2] = t2
            with tc.tile_pool(name="pst2", bufs=2, space="PSUM") as pst2:
                ps_sum2 = pst2.tile([1, TQ], F32, name="ps_sum2", tag="st")
                ps_sq2 = pst2.tile([1, TQ], F32, name="ps_sq2", tag="st")
                for mc in range(CC):
                    nc.vector.scalar_tensor_tensor(
                        r2_sb[mc][:], ps_m[mc][:], prm(mc, P_B2),
                        hT_sb[mc][:], mybir.AluOpType.add,
                        mybir.AluOpType.add)
                    nc.tensor.matmul(ps_sum2[:], ones_sb[:, 0:1], r2_sb[mc][:],
                                     start=(mc == 0), stop=(mc == CC - 1))
                    sq = dpool.tile([128, TQ], F32R, name="sq2", tag="sq")
                    nc.scalar.activation(sq[:], r2_sb[mc][:], ACTF.Square)
                    nc.tensor.matmul(ps_sq2[:], ones_sb[:, 0:1], sq[:],
                                     start=(mc == 0), stop=(mc == CC - 1))
                n = float(C)
                mean2 = dpool.tile([1, TQ], F32R, name="l2mean", tag="l2mean")
                nc.scalar.activation(mean2[:], ps_sum2[:], ACTF.Copy,
                                     scale=1.0 / n)
                ex22 = dpool.tile([1, TQ], F32, name="l2ex2", tag="l2ex2")
                nc.scalar.activation(ex22[:], ps_sq2[:], ACTF.Copy,
                                     scale=1.0 / n)
        hT_holder["pool"].release()
        _ln_bcast(nc, dpool, eps_sb, lnr1b, lnr2b, mean2, ex22, "l2")
        with tc.tile_pool(name="pbc3", bufs=2, space="PSUM") as pbc3:
            for cc in range(CC):
                csl = slice(cc * 128, (cc + 1) * 128)
                pb = pbc3.tile([128, 2 * TQ], F32, name="lnpb3", tag="bc")
                nc.tensor.matmul(pb[:, 0:TQ], lnt2_sb[:, csl],
                                 lnr1b[:], start=True, stop=True)
                nc.tensor.matmul(pb[:, TQ:], lnt2_sb[:, csl],
                                 lnr2b[:], start=True, stop=True)
                t1 = dpool.tile([128, TQ], F32, name="ln_t13", tag="lnt13")
                nc.vector.tensor_mul(t1[:], r2_sb[cc][:], pb[:, 0:TQ])
                ot = dpool.tile([128, TQ], F32, name=f"o{cc}", tag=f"o{cc}",
                                bufs=1)
                nc.vector.tensor_sub(ot[:], t1[:], pb[:, TQ:])
                nc.sync.dma_start(out=d["outT"][cc * 128:(cc + 1) * 128, :],
                                  in_=ot[:])

    w2pool.release()
    w1pool.release()
    wo_pool.release()
    ctxT_pool.release()
    const.release()


_NC = None


def _get_nc():
    global _NC
    if _NC is None:
        _NC = _build_nc()
    return _NC


def _prep_inmaps(x, Wq, bq, Wk, bk, Wv, bv, Wo, bo, ln1_s, ln1_b,
                 W1, b1, W2, b2, ln2_s, ln2_b):
    f32 = np.float32
    bf16 = ml_dtypes.bfloat16

    def pk(a):
        # [A*128, c...] -> [128, A, c...] contiguous (partition-major)
        a = np.asarray(a)
        return np.ascontiguousarray(
            a.reshape(-1, 128, *a.shape[1:]).swapaxes(0, 1))

    xT = [np.ascontiguousarray(np.asarray(x)[b].T, dtype=f32)
          for b in range(B)]
    xTh = [xb.astype(bf16) for xb in xT]
    wq = pk(np.asarray(Wq, f32).astype(bf16))
    wk = pk(np.asarray(Wk, f32).astype(bf16))
    wv = pk(np.asarray(Wv, f32).astype(bf16))
    wo = pk(np.asarray(Wo, f32).astype(bf16))
    w1f = np.asarray(W1, f32).astype(bf16)
    w1 = np.stack([pk(w1f[:, jb * C:(jb + 1) * C]) for jb in range(4)],
                  axis=1)
    w2f = np.asarray(W2, f32).astype(bf16)
    w2 = np.stack([pk(w2f[gg * C:(gg + 1) * C, :]) for gg in range(4)],
                  axis=1)
    prk = np.zeros((128, CC, 8), f32)
    for pi, arr in ((P_BQ, bq), (P_BK, bk), (P_BO, bo), (P_B2, b2),
                    (P_L1S, ln1_s), (P_L1B, ln1_b), (P_L2S, ln2_s),
                    (P_L2B, ln2_b)):
        prk[:, :, pi] = np.asarray(arr, f32).reshape(CC, 128).T
    b1p = np.ascontiguousarray(np.asarray(b1, f32).reshape(ICN, 128).T)
    bvb = np.broadcast_to(np.asarray(bv, f32)[None, :], (128, C)).copy()
    ones = np.ones((128, TQ), f32)
    lnt = np.zeros((2, 65, C), f32)
    lnt[0, 0], lnt[0, 32] = np.asarray(ln1_s, f32), -np.asarray(ln1_b, f32)
    lnt[1, 0], lnt[1, 32] = np.asarray(ln2_s, f32), -np.asarray(ln2_b, f32)
    selm = np.zeros((65, 3, 128), f32)
    for j in range(3):
        selm[32 * j, j, :] = 1.0
    lnz = np.zeros((2, 65, TQ), f32)
    lnz[1, 32, :] = 1.0

    kk = np.arange(128)[:, None]
    in_maps = []
    for c in range(8):
        b, p = c // 4, c % 4

        def mk(kc, qq0, w):
            qq = (np.arange(w) + qq0)[None, :]
            return (128 * kc + kk <= 4 * qq + p).astype(bf16)

        msk = np.ascontiguousarray(mk(0, 0, 32))  # k <= 4j+p, all chunks

        xtp = pk(xTh[b])  # [128, CC, T]
        xtp = np.ascontiguousarray(
            xtp.reshape(128, CC, TBN, 512).swapaxes(1, 2))
        in_maps.append({
            "xt": xtp,
            "xtqh": pk(np.ascontiguousarray(xTh[b][:, p::4])),
            "xtq": pk(np.ascontiguousarray(xT[b][:, p::4])),
            "wq": wq, "wk": wk, "wv": wv, "wo": wo, "w1": w1, "w2": w2,
            "prk": prk, "b1p": b1p, "bvb": bvb, "msk": msk,
            "ones": ones, "lnt": lnt, "selm": selm, "lnz": lnz,
        })
    return in_maps


def _run(in_maps, trace=False, **kw):
    nc = _get_nc()
    return run_bass_kernel_spmd(nc, in_maps, list(range(8)), trace=trace, **kw)


def kernel(**inputs):
    in_maps = _prep_inmaps(**inputs)
    res = _run(in_maps)
    out = np.empty((B, T, C), np.float32)
    for c in range(8):
        b, p = c // 4, c % 4
        out[b, p::4, :] = res.results[c]["outT"].T
    return out
